# revision 17
# baseline (speedup 1.0000x reference)
"""Cross-attention Trainium2 Bass kernel (v2).

Full inputs in, full output out. Internally: 8-way sharding, data-parallel
over batch (B=2) x tensor-parallel over head groups (16 heads -> 4 groups
of 4). Core c handles batch c//4, head group c%4. Each core computes a
partial output (its 4 heads' contribution through W_o); the host sums the
4 partials per batch and adds b_o.

Device-side structure (v2):
  - All matmuls bf16 (fp32 PSUM accumulate); output DMA'd as fp16 partials.
  - Activations fed pre-transposed (D on partitions).
  - S^T tiles [k x q] via FULL 128-contraction matmuls: the K-stationary
    for each head is zero-padded to 128 rows (kTz0 top-real/bottom-zero,
    kTz1 bottom-real/top-zero) so the moving operand is the shared qT tile.
  - exp batched 2 PSUM banks per ACTIVATE ([128, 2, 512]) to amortize the
    ~352-cycle ACT pipeline overhead.
  - A@V computed as Y^T = V'^T @ P^T: stationary = V' chunk [128k, 65]
    (65th col = ones -> softmax denominators in row 64), moving = exp'd
    score tile [128k, 512q]. Output lands directly in Y^T orientation,
    so no PE transposes. Normalization: DVE reciprocal of the denominator
    row, rank-1 matmul broadcast into partitions 64:128 of the same PSUM
    bank, DVE multiply -> yT (bf16).
  - g-major block loop with just-in-time Q projections; O projection and
    fp16 output DMA interleaved after each q-group.
"""

import numpy as np
import ml_dtypes

T = 2048          # T_dec == T_enc
D = 1024          # d_model
P = 128
HPC = 4           # heads per core
DH = 64           # head dim
KC = D // P       # 8 contraction chunks for projections
NKT = T // P      # 16 key chunks
QG = 512          # q-group width
NQG = T // QG     # 4 q groups
NCORES = 8

_PROGRAM = None


def _split_multiwaits_json(raw: bytes) -> bytes:
    """This walrus build accepts at most ONE sync-wait per instruction.
    Split every multi-wait instruction into single-wait same-engine NoOps
    followed by the instruction (same-engine program order preserves
    semantics exactly)."""
    try:
        import orjson as _json

        loads, dumps = _json.loads, _json.dumps
    except ImportError:
        import json as _json

        loads = _json.loads
        dumps = lambda o: _json.dumps(o).encode()

    j = loads(raw)
    k = 0
    for fn in j["functions"]:
        for bb in fn["blocks"]:
            insts = bb["instructions"]
            out = []
            changed = False
            for inst in insts:
                si = inst.get("sync_info")
                waits = (si.get("on_wait") or []) if si else []
                if len(waits) > 1:
                    for w in waits[:-1]:
                        nop = {
                            "engine": inst["engine"],
                            "ins": [],
                            "outs": [],
                            "name": f"{inst['name']}-sw{k}",
                            "opcode": "NoOp",
                            "sync_info": {"on_update": [], "on_wait": [w]},
                        }
                        if inst.get("debug") is not None:
                            nop["debug"] = inst["debug"]
                        out.append(nop)
                        k += 1
                    si["on_wait"] = [waits[-1]]
                    changed = True
                out.append(inst)
            if changed:
                bb["instructions"] = out
    return dumps(j)


def _build_program():
    import concourse.bass as bass
    import concourse.tile as tile
    import concourse.mybir as mybir
    from concourse.vector_clock import ScopedClock
    from contextlib import ExitStack

    f32 = mybir.dt.float32
    f16 = mybir.dt.float16
    bf16 = mybir.dt.bfloat16
    EXP = mybir.ActivationFunctionType.Exp

    class _TC(tile.TileContext):
        # This walrus build rejects >1 sync waits on the CTRL Drain
        # encoding; split the kernel-tail drain's waits into single-wait
        # SP instructions instead.
        def _drain_and_barrier(self, tick_clock, wait_clock):
            dummy = mybir.InstNoOp(
                name="wait-collector", engine=mybir.EngineType.SP
            )
            wait_clock.add_sem_waits(
                dummy, ScopedClock({None: tick_clock.global_clock})
            )
            si = dummy.sync_info
            waits = list(si.on_wait) if si and si.on_wait else []
            assert self.sems is not None
            by_name = {h.name: h for h in self.sems.allocated().values()}
            for w in waits:
                self.nc.sync.wait_ge(by_name[w.ant_name], w.wait_value)
            self.nc.sync.drain()
            self.nc.all_engine_barrier()
            popped = self.nc._tile_sem_poison_stack.pop()
            assert popped is self._sem_poison
            self.nc.clear_and_free_semaphores(
                list(self.sems.allocated().values())
            )
            self.nc.all_engine_barrier()

    nc = bass.Bass()

    tgtT = nc.dram_tensor("tgtT", [D, T], bf16, kind="ExternalInput")
    memT = nc.dram_tensor("memT", [D, T], bf16, kind="ExternalInput")
    wq = nc.dram_tensor("wq", [D, 256], bf16, kind="ExternalInput")
    wk = nc.dram_tensor("wk", [D, 256], bf16, kind="ExternalInput")
    wv = nc.dram_tensor("wv", [D, 256], bf16, kind="ExternalInput")
    wo = nc.dram_tensor("wo", [256, D], bf16, kind="ExternalInput")
    bq = nc.dram_tensor("bq", [256], f32, kind="ExternalInput")
    bk = nc.dram_tensor("bk", [256], f32, kind="ExternalInput")
    bv = nc.dram_tensor("bv", [256], f32, kind="ExternalInput")
    out = nc.dram_tensor("out", [T, D], f16, kind="ExternalOutput")

    with _TC(nc) as tc, ExitStack() as ctx:
        singles = ctx.enter_context(tc.tile_pool(name="singles", bufs=1))

        # ---- stage A: load weights + transposed activations ----
        # DMA issue order matters for startup: wk+mem first (K/V proj gate
        # the first attention block), tgt + wq later, wo last.
        wk_s = singles.tile([P, KC, 256], bf16, tag="wk")
        # mT/tT as per-q-group quarter tiles so compute deps are quarter-
        # granular (the tile framework tracks writer->reader deps per tile).
        mTq = [
            singles.tile([P, KC, QG], bf16, tag=f"mTq{i}", name=f"mTq{i}")
            for i in range(NQG)
        ]
        wv_s = singles.tile([P, KC, 256], bf16, tag="wv")
        tTq = [
            singles.tile([P, KC, QG], bf16, tag=f"tTq{i}", name=f"tTq{i}")
            for i in range(NQG)
        ]
        wq_s = singles.tile([P, KC, 256], bf16, tag="wq")
        wo_s = singles.tile([P, 2, D], bf16, tag="wo")

        bq_s = singles.tile([P, 2], f32, tag="bq")
        bk_s = singles.tile([P, 2], f32, tag="bk")
        bvb = singles.tile([P, 256], f32, tag="bvb")

        memT_r = memT.rearrange("(c p) t -> p c t", p=P)
        tgtT_r = tgtT.rearrange("(c p) t -> p c t", p=P)

        # Critical loads on the sync-engine HWDGE queue.
        nc.sync.dma_start(wk_s, wk.rearrange("(c p) n -> p c n", p=P))
        nc.sync.dma_start(bk_s, bk.rearrange("(c p) -> p c", p=P))
        for i in range(NQG):
            nc.sync.dma_start(mTq[i], memT_r[:, :, i * QG : (i + 1) * QG])
        nc.sync.dma_start(wv_s, wv.rearrange("(c p) n -> p c n", p=P))
        bv_ap = bass.AP(tensor=bv[:].tensor, offset=0, ap=[[0, P], [1, 256]])
        nc.sync.dma_start(bvb, bv_ap)
        # Deprioritized loads: issued from the scalar engine AFTER a dummy
        # read of mT's last quarter, so they don't steal HBM bandwidth from
        # the loads gating the first matmuls.
        dgate = singles.tile([1, 8], bf16, tag="dgate")
        nc.scalar.copy(dgate, mTq[3][0:1, 0, 0:8])
        for i in range(NQG):
            nc.scalar.dma_start(tTq[i], tgtT_r[:, :, i * QG : (i + 1) * QG])
            if i == 0:
                nc.scalar.dma_start(
                    wq_s, wq.rearrange("(c p) n -> p c n", p=P)
                )
                nc.scalar.dma_start(bq_s, bq.rearrange("(c p) -> p c", p=P))
        nc.scalar.dma_start(wo_s, wo.rearrange("(c p) n -> p c n", p=P))

        # persistent intermediates
        qT = singles.tile([P, 2, T], bf16, tag="qT")    # Q^T (head-pair rows)
        # zero-padded K stationaries: kTz0 rows 0:64 = head0 K^T, rows
        # 64:128 = 0; kTz1 rows 0:64 = 0, rows 64:128 = head1 K^T.
        kTz0 = singles.tile([P, 2, T], bf16, tag="kTz0")
        kTz1 = singles.tile([P, 2, T], bf16, tag="kTz1")
        # V' per key-chunk: [V|1] for every head (k on partitions)
        vS = singles.tile([P, NKT, HPC, DH + 1], bf16, tag="vS")
        yT = singles.tile([P, 2, T], bf16, tag="yT")    # normalized Y^T

        ones1 = singles.tile([1, DH], f16, tag="ones1")  # rank-1 stationary
        dsb0 = singles.tile([1, QG], f16, tag="dsb0")    # den row h0 (f16)
        dsb1 = singles.tile([1, QG], f16, tag="dsb1")    # den row h1 (f16)
        rbc = singles.tile([DH, QG], f32, tag="rbc")     # 1/den broadcast

        nc.vector.memset(vS[:, :, :, DH : DH + 1], 1.0)
        nc.vector.memset(kTz0[64:128, :, :], 0.0)
        nc.vector.memset(kTz1[0:64, :, :], 0.0)
        nc.vector.memset(ones1, 1.0)

        # ---- prologue: all projections (own PSUM pool, closed after) ----
        with tc.tile_pool(name="apsum", bufs=4, space="PSUM") as apsum:

            def k_proj(mc, g):
                cols = slice(g * QG, (g + 1) * QG)
                pk = apsum.tile([P, QG], f32, tag="pa")
                for c in range(KC):
                    nc.tensor.matmul(
                        pk,
                        wk_s[:, c, mc * P : (mc + 1) * P],
                        mTq[g][:, c, :],
                        start=(c == 0),
                        stop=(c == KC - 1),
                    )
                nc.vector.tensor_scalar_add(
                    kTz0[0:64, mc, cols], pk[0:64, :], bk_s[0:64, mc : mc + 1]
                )
                nc.vector.tensor_scalar_add(
                    kTz1[64:128, mc, cols],
                    pk[64:128, :],
                    bk_s[64:128, mc : mc + 1],
                )

            def q_proj(mc, g):
                cols = slice(g * QG, (g + 1) * QG)
                pq = apsum.tile([P, QG], f32, tag="pa")
                for c in range(KC):
                    nc.tensor.matmul(
                        pq,
                        wq_s[:, c, mc * P : (mc + 1) * P],
                        tTq[g][:, c, :],
                        start=(c == 0),
                        stop=(c == KC - 1),
                    )
                nc.vector.tensor_scalar_add(
                    qT[:, mc, cols], pq, bq_s[:, mc : mc + 1]
                )

            def v_proj(tt):
                pv = apsum.tile([P, QG], f32, tag="pa")
                for c in range(KC):
                    nc.tensor.matmul(
                        pv[:, 0:256],
                        mTq[tt // 4][:, c, (tt % 4) * P : (tt % 4 + 1) * P],
                        wv_s[:, c, :],
                        start=(c == 0),
                        stop=(c == KC - 1),
                    )
                pv4 = pv[:, 0:256].rearrange("p (h d) -> p h d", h=HPC)
                bvb4 = bvb.rearrange("p (h d) -> p h d", h=HPC)
                nc.vector.tensor_add(vS[:, tt, :, 0:DH], pv4, bvb4)

            # K/V interleaved with mT quarter arrival.
            k_proj(0, 0)
            k_proj(0, 1)
            for tt in range(8):
                v_proj(tt)
            k_proj(0, 2)
            k_proj(0, 3)
            for tt in range(8, NKT):
                v_proj(tt)
            for g in range(NQG):
                k_proj(1, g)
            for g in range(NQG):
                for mc in range(2):
                    q_proj(mc, g)

        # ---- attention (prologue PSUM banks freed: 4+4 of 8 used) ----
        with (
            tc.tile_pool(name="spsum", bufs=2, space="PSUM") as spsum,
            tc.tile_pool(name="ypsum", bufs=4, space="PSUM") as ypsum,
            tc.tile_pool(name="ostg", bufs=3) as ostg,
            tc.tile_pool(name="ptp", bufs=6) as ptp,
        ):

            def o_tt(tt):
                qrows = slice(tt * P, (tt + 1) * P)
                po = spsum.tile([P, 2, QG], f32, tag="s")
                for ng in range(2):
                    ncols = slice(ng * QG, (ng + 1) * QG)
                    for mc in range(2):
                        nc.tensor.matmul(
                            po[:, ng, :],
                            yT[:, mc, qrows],
                            wo_s[:, mc, ncols],
                            start=(mc == 0),
                            stop=(mc == 1),
                        )
                og = ostg.tile([P, D], f16, tag="og")
                nc.vector.tensor_copy(og[:, 0:QG], po[:, 0, :])
                nc.vector.tensor_copy(og[:, QG:D], po[:, 1, :])
                nc.sync.dma_start(out[qrows, :], og)

            # ---- attention blocks, g-major ----
            def block(g, mc, o_tts):
                cols = slice(g * QG, (g + 1) * QG)
                psY = [
                    ypsum.tile([P, QG], f32, tag="py", name="psY0"),
                    ypsum.tile([P, QG], f32, tag="py", name="psY1"),
                ]
                pts = [None] * NKT

                def s_pair(kc):
                    krows = slice(kc * P, (kc + 1) * P)
                    psS = spsum.tile([P, 2, QG], f32, tag="s")
                    nc.tensor.matmul(
                        psS[:, 0, :],
                        kTz0[:, mc, krows],
                        qT[:, mc, cols],
                        start=True,
                        stop=True,
                    )
                    nc.tensor.matmul(
                        psS[:, 1, :],
                        kTz1[:, mc, krows],
                        qT[:, mc, cols],
                        start=True,
                        stop=True,
                    )
                    pt = ptp.tile([P, 2, QG], bf16, tag="pt")
                    nc.scalar.activation(pt, psS, EXP, scale=0.125)
                    pts[kc] = pt

                def av_pair(kc):
                    for hh in range(2):
                        nc.tensor.matmul(
                            psY[hh][0:65, :],
                            vS[:, kc, 2 * mc + hh, :],
                            pts[kc][:, hh, :],
                            start=(kc == 0),
                            stop=(kc == NKT - 1),
                        )

                # O-projection row-tiles of the previous q-group are slotted
                # into this block's kc loop (their yT is long since final) so
                # they fill tensor slack without starving the ACT pipeline.
                for kc in range(NKT):
                    s_pair(kc)
                    if kc > 0:
                        av_pair(kc - 1)
                    if kc in (5, 11) and o_tts:
                        o_tt(o_tts.pop(0))
                av_pair(NKT - 1)

                # den rows (partition 64) -> f16 -> rank-1 broadcast into
                # partitions 64:128 of the same bank -> reciprocal -> multiply.
                # Casts first so the tensor-queue rank-1s release quickly.
                dsb = (dsb0, dsb1)
                with nc.allow_low_precision(
                    reason="den in fp16 for the rank-1 broadcast; "
                    "den is O(1e2..1e4), fp16 rel err ~5e-4"
                ):
                    nc.vector.tensor_copy(dsb0, psY[0][64:65, :])
                    nc.vector.tensor_copy(dsb1, psY[1][64:65, :])
                for hh in range(2):
                    nc.tensor.matmul(
                        psY[hh][64:128, :],
                        ones1,
                        dsb[hh],
                        start=True,
                        stop=True,
                        skip_group_check=True,
                    )
                for hh in range(2):
                    nc.vector.reciprocal(rbc, psY[hh][64:128, :])
                    nc.vector.tensor_mul(
                        yT[hh * DH : (hh + 1) * DH, mc, cols],
                        psY[hh][0:64, :],
                        rbc,
                    )

            for g in range(NQG):
                pend = list(range(4 * (g - 1), 4 * g)) if g > 0 else []
                for mc in range(2):
                    block(g, mc, pend)
            for tt in range(12, 16):
                o_tt(tt)

    _orig_to_json = nc.to_json_bytes
    nc.to_json_bytes = lambda: _split_multiwaits_json(_orig_to_json())
    return nc


def _get_program():
    global _PROGRAM
    if _PROGRAM is None:
        _PROGRAM = _build_program()
    return _PROGRAM


def _in_maps(tgt, memory, W_q, b_q, W_k, b_k, W_v, b_v, W_o):
    bf16 = ml_dtypes.bfloat16
    maps = []
    tT = [np.ascontiguousarray(tgt[b].T).astype(bf16) for b in range(2)]
    mT = [np.ascontiguousarray(memory[b].T).astype(bf16) for b in range(2)]
    for c in range(NCORES):
        b, hg = c // HPC, c % HPC
        sl = slice(hg * 256, (hg + 1) * 256)
        maps.append(
            {
                "tgtT": tT[b],
                "memT": mT[b],
                "wq": np.ascontiguousarray(W_q[:, sl]).astype(bf16),
                "wk": np.ascontiguousarray(W_k[:, sl]).astype(bf16),
                "wv": np.ascontiguousarray(W_v[:, sl]).astype(bf16),
                "wo": np.ascontiguousarray(W_o[sl, :]).astype(bf16),
                "bq": np.ascontiguousarray(b_q[sl]).astype(np.float32),
                "bk": np.ascontiguousarray(b_k[sl]).astype(np.float32),
                "bv": np.ascontiguousarray(b_v[sl]).astype(np.float32),
            }
        )
    return maps


def kernel(tgt, memory, W_q, b_q, W_k, b_k, W_v, b_v, W_o, b_o):
    from concourse.bass_utils import run_bass_kernel_spmd

    tgt = np.asarray(tgt)
    memory = np.asarray(memory)
    nc = _get_program()
    maps = _in_maps(
        np.asarray(tgt, np.float32),
        np.asarray(memory, np.float32),
        np.asarray(W_q, np.float32),
        np.asarray(b_q, np.float32),
        np.asarray(W_k, np.float32),
        np.asarray(b_k, np.float32),
        np.asarray(W_v, np.float32),
        np.asarray(b_v, np.float32),
        np.asarray(W_o, np.float32),
    )
    res = run_bass_kernel_spmd(nc, maps, core_ids=list(range(NCORES)))
    outs = [r["out"] for r in res.results]
    bo = np.asarray(b_o, np.float64)
    full = np.empty((2, T, D), np.float32)
    for b in range(2):
        acc = np.zeros((T, D), np.float64)
        for hg in range(HPC):
            acc += outs[b * HPC + hg].astype(np.float64)
        full[b] = (acc + bo).astype(np.float32)
    return full


# revision 21
# speedup vs baseline: 1.0033x; 1.0033x over previous
"""Cross-attention Trainium2 Bass kernel (v2).

Full inputs in, full output out. Internally: 8-way sharding, data-parallel
over batch (B=2) x tensor-parallel over head groups (16 heads -> 4 groups
of 4). Core c handles batch c//4, head group c%4. Each core computes a
partial output (its 4 heads' contribution through W_o); the host sums the
4 partials per batch and adds b_o.

Device-side structure (v2):
  - All matmuls bf16 (fp32 PSUM accumulate); output DMA'd as fp16 partials.
  - Activations fed pre-transposed (D on partitions).
  - S^T tiles [k x q] via FULL 128-contraction matmuls: the K-stationary
    for each head is zero-padded to 128 rows (kTz0 top-real/bottom-zero,
    kTz1 bottom-real/top-zero) so the moving operand is the shared qT tile.
  - exp batched 2 PSUM banks per ACTIVATE ([128, 2, 512]) to amortize the
    ~352-cycle ACT pipeline overhead.
  - A@V computed as Y^T = V'^T @ P^T: stationary = V' chunk [128k, 65]
    (65th col = ones -> softmax denominators in row 64), moving = exp'd
    score tile [128k, 512q]. Output lands directly in Y^T orientation,
    so no PE transposes. Normalization: DVE reciprocal of the denominator
    row, rank-1 matmul broadcast into partitions 64:128 of the same PSUM
    bank, DVE multiply -> yT (bf16).
  - g-major block loop with just-in-time Q projections; O projection and
    fp16 output DMA interleaved after each q-group.
"""

import numpy as np
import ml_dtypes

T = 2048          # T_dec == T_enc
D = 1024          # d_model
P = 128
HPC = 4           # heads per core
DH = 64           # head dim
KC = D // P       # 8 contraction chunks for projections
NKT = T // P      # 16 key chunks
QG = 512          # q-group width
NQG = T // QG     # 4 q groups
NCORES = 8

_PROGRAM = None


def _split_multiwaits_json(raw: bytes) -> bytes:
    """This walrus build accepts at most ONE sync-wait per instruction.
    Split every multi-wait instruction into single-wait same-engine NoOps
    followed by the instruction (same-engine program order preserves
    semantics exactly)."""
    try:
        import orjson as _json

        loads, dumps = _json.loads, _json.dumps
    except ImportError:
        import json as _json

        loads = _json.loads
        dumps = lambda o: _json.dumps(o).encode()

    j = loads(raw)
    k = 0
    for fn in j["functions"]:
        for bb in fn["blocks"]:
            insts = bb["instructions"]
            out = []
            changed = False
            for inst in insts:
                si = inst.get("sync_info")
                waits = (si.get("on_wait") or []) if si else []
                if len(waits) > 1:
                    for w in waits[:-1]:
                        nop = {
                            "engine": inst["engine"],
                            "ins": [],
                            "outs": [],
                            "name": f"{inst['name']}-sw{k}",
                            "opcode": "NoOp",
                            "sync_info": {"on_update": [], "on_wait": [w]},
                        }
                        if inst.get("debug") is not None:
                            nop["debug"] = inst["debug"]
                        out.append(nop)
                        k += 1
                    si["on_wait"] = [waits[-1]]
                    changed = True
                out.append(inst)
            if changed:
                bb["instructions"] = out
    return dumps(j)


def _build_program():
    import concourse.bass as bass
    import concourse.tile as tile
    import concourse.mybir as mybir
    from concourse.vector_clock import ScopedClock
    from contextlib import ExitStack

    f32 = mybir.dt.float32
    f16 = mybir.dt.float16
    bf16 = mybir.dt.bfloat16
    EXP = mybir.ActivationFunctionType.Exp

    class _TC(tile.TileContext):
        # This walrus build rejects >1 sync waits on the CTRL Drain
        # encoding; split the kernel-tail drain's waits into single-wait
        # SP instructions instead.
        def _drain_and_barrier(self, tick_clock, wait_clock):
            dummy = mybir.InstNoOp(
                name="wait-collector", engine=mybir.EngineType.SP
            )
            wait_clock.add_sem_waits(
                dummy, ScopedClock({None: tick_clock.global_clock})
            )
            si = dummy.sync_info
            waits = list(si.on_wait) if si and si.on_wait else []
            assert self.sems is not None
            by_name = {h.name: h for h in self.sems.allocated().values()}
            for w in waits:
                self.nc.sync.wait_ge(by_name[w.ant_name], w.wait_value)
            self.nc.sync.drain()
            self.nc.all_engine_barrier()
            popped = self.nc._tile_sem_poison_stack.pop()
            assert popped is self._sem_poison
            self.nc.clear_and_free_semaphores(
                list(self.sems.allocated().values())
            )
            self.nc.all_engine_barrier()

    nc = bass.Bass()

    tgtT = nc.dram_tensor("tgtT", [D, T], bf16, kind="ExternalInput")
    memT = nc.dram_tensor("memT", [D, T], bf16, kind="ExternalInput")
    wq = nc.dram_tensor("wq", [D, 256], bf16, kind="ExternalInput")
    wk = nc.dram_tensor("wk", [D, 256], bf16, kind="ExternalInput")
    wv = nc.dram_tensor("wv", [D, 256], bf16, kind="ExternalInput")
    wo = nc.dram_tensor("wo", [256, D], bf16, kind="ExternalInput")
    bq = nc.dram_tensor("bq", [256], f32, kind="ExternalInput")
    bk = nc.dram_tensor("bk", [256], f32, kind="ExternalInput")
    bv = nc.dram_tensor("bv", [256], f32, kind="ExternalInput")
    out = nc.dram_tensor("out", [T, D], f16, kind="ExternalOutput")

    with _TC(nc) as tc, ExitStack() as ctx:
        singles = ctx.enter_context(tc.tile_pool(name="singles", bufs=1))

        # ---- stage A: load weights + transposed activations ----
        # DMA issue order matters for startup: wk+mem first (K/V proj gate
        # the first attention block), tgt + wq later, wo last.
        wk_s = singles.tile([P, KC, 256], bf16, tag="wk")
        # mT/tT as per-q-group quarter tiles so compute deps are quarter-
        # granular (the tile framework tracks writer->reader deps per tile).
        mTq = [
            singles.tile([P, KC, QG], bf16, tag=f"mTq{i}", name=f"mTq{i}")
            for i in range(NQG)
        ]
        wv_s = singles.tile([P, KC, 256], bf16, tag="wv")
        tTq = [
            singles.tile([P, KC, QG], bf16, tag=f"tTq{i}", name=f"tTq{i}")
            for i in range(NQG)
        ]
        wq_s = singles.tile([P, KC, 256], bf16, tag="wq")
        wo_s = singles.tile([P, 2, D], bf16, tag="wo")

        bq_s = singles.tile([P, 2], f32, tag="bq")
        bk_s = singles.tile([P, 2], f32, tag="bk")
        bvb = singles.tile([P, 256], f32, tag="bvb")

        memT_r = memT.rearrange("(c p) t -> p c t", p=P)
        tgtT_r = tgtT.rearrange("(c p) t -> p c t", p=P)

        # Critical loads on the sync-engine HWDGE queue.
        nc.sync.dma_start(wk_s, wk.rearrange("(c p) n -> p c n", p=P))
        nc.sync.dma_start(bk_s, bk.rearrange("(c p) -> p c", p=P))
        for i in range(NQG):
            nc.sync.dma_start(mTq[i], memT_r[:, :, i * QG : (i + 1) * QG])
        nc.sync.dma_start(wv_s, wv.rearrange("(c p) n -> p c n", p=P))
        bv_ap = bass.AP(tensor=bv[:].tensor, offset=0, ap=[[0, P], [1, 256]])
        nc.sync.dma_start(bvb, bv_ap)
        # Deprioritized loads: issued from the scalar engine AFTER a dummy
        # read of mT's last quarter, so they don't steal HBM bandwidth from
        # the loads gating the first matmuls.
        dgate = singles.tile([1, 8], bf16, tag="dgate")
        nc.scalar.copy(dgate, mTq[3][0:1, 0, 0:8])
        for i in range(NQG):
            nc.scalar.dma_start(tTq[i], tgtT_r[:, :, i * QG : (i + 1) * QG])
            if i == 0:
                nc.scalar.dma_start(
                    wq_s, wq.rearrange("(c p) n -> p c n", p=P)
                )
                nc.scalar.dma_start(bq_s, bq.rearrange("(c p) -> p c", p=P))
        nc.scalar.dma_start(wo_s, wo.rearrange("(c p) n -> p c n", p=P))

        # persistent intermediates
        qT = singles.tile([P, 2, T], bf16, tag="qT")    # Q^T (head-pair rows)
        # zero-padded K stationaries: kTz0 rows 0:64 = head0 K^T, rows
        # 64:128 = 0; kTz1 rows 0:64 = 0, rows 64:128 = head1 K^T.
        kTz0 = singles.tile([P, 2, T], bf16, tag="kTz0")
        kTz1 = singles.tile([P, 2, T], bf16, tag="kTz1")
        # V' per key-chunk: [V|1] for every head (k on partitions)
        vS = singles.tile([P, NKT, HPC, DH + 1], bf16, tag="vS")
        # normalized Y^T, split per q-group so O-projection reads of group
        # g-1 don't serialize against normalize writes of group g.
        yTg = [
            singles.tile([P, 2, QG], bf16, tag=f"yTg{i}", name=f"yTg{i}")
            for i in range(NQG)
        ]

        ones1 = singles.tile([1, DH], f16, tag="ones1")  # rank-1 stationary
        dsb0 = singles.tile([1, QG], f16, tag="dsb0")    # den row h0 (f16)
        dsb1 = singles.tile([1, QG], f16, tag="dsb1")    # den row h1 (f16)
        rbc = singles.tile([DH, QG], f32, tag="rbc")     # 1/den broadcast

        nc.vector.memset(vS[:, :, :, DH : DH + 1], 1.0)
        nc.vector.memset(kTz0[64:128, :, :], 0.0)
        nc.vector.memset(kTz1[0:64, :, :], 0.0)
        nc.vector.memset(ones1, 1.0)

        # ---- prologue: all projections (own PSUM pool, closed after) ----
        with tc.tile_pool(name="apsum", bufs=4, space="PSUM") as apsum:

            def k_proj(mc, g):
                cols = slice(g * QG, (g + 1) * QG)
                pk = apsum.tile([P, QG], f32, tag="pa")
                for c in range(KC):
                    nc.tensor.matmul(
                        pk,
                        wk_s[:, c, mc * P : (mc + 1) * P],
                        mTq[g][:, c, :],
                        start=(c == 0),
                        stop=(c == KC - 1),
                    )
                nc.vector.tensor_scalar_add(
                    kTz0[0:64, mc, cols], pk[0:64, :], bk_s[0:64, mc : mc + 1]
                )
                nc.vector.tensor_scalar_add(
                    kTz1[64:128, mc, cols],
                    pk[64:128, :],
                    bk_s[64:128, mc : mc + 1],
                )

            def q_proj(mc, g):
                cols = slice(g * QG, (g + 1) * QG)
                pq = apsum.tile([P, QG], f32, tag="pa")
                for c in range(KC):
                    nc.tensor.matmul(
                        pq,
                        wq_s[:, c, mc * P : (mc + 1) * P],
                        tTq[g][:, c, :],
                        start=(c == 0),
                        stop=(c == KC - 1),
                    )
                nc.vector.tensor_scalar_add(
                    qT[:, mc, cols], pq, bq_s[:, mc : mc + 1]
                )

            def v_proj(tt):
                pv = apsum.tile([P, QG], f32, tag="pa")
                for c in range(KC):
                    nc.tensor.matmul(
                        pv[:, 0:256],
                        mTq[tt // 4][:, c, (tt % 4) * P : (tt % 4 + 1) * P],
                        wv_s[:, c, :],
                        start=(c == 0),
                        stop=(c == KC - 1),
                    )
                pv4 = pv[:, 0:256].rearrange("p (h d) -> p h d", h=HPC)
                bvb4 = bvb.rearrange("p (h d) -> p h d", h=HPC)
                nc.vector.tensor_add(vS[:, tt, :, 0:DH], pv4, bvb4)

            # K/V interleaved with mT quarter arrival.
            k_proj(0, 0)
            k_proj(0, 1)
            for tt in range(8):
                v_proj(tt)
            k_proj(0, 2)
            k_proj(0, 3)
            for tt in range(8, NKT):
                v_proj(tt)
            for g in range(NQG):
                k_proj(1, g)
            for g in range(NQG):
                for mc in range(2):
                    q_proj(mc, g)

        # ---- attention (prologue PSUM banks freed: 4+4 of 8 used) ----
        with (
            tc.tile_pool(name="spsum", bufs=2, space="PSUM") as spsum,
            tc.tile_pool(name="ypsum", bufs=4, space="PSUM") as ypsum,
            tc.tile_pool(name="ostg", bufs=3) as ostg,
            tc.tile_pool(name="ptp", bufs=6) as ptp,
        ):

            def o_tt(tt):
                qrows = slice((tt % 4) * P, (tt % 4 + 1) * P)
                grows = slice(tt * P, (tt + 1) * P)
                po = spsum.tile([P, 2, QG], f32, tag="s")
                for ng in range(2):
                    ncols = slice(ng * QG, (ng + 1) * QG)
                    for mc in range(2):
                        nc.tensor.matmul(
                            po[:, ng, :],
                            yTg[tt // 4][:, mc, qrows],
                            wo_s[:, mc, ncols],
                            start=(mc == 0),
                            stop=(mc == 1),
                        )
                og = ostg.tile([P, D], f16, tag="og")
                nc.vector.tensor_copy(og[:, 0:QG], po[:, 0, :])
                nc.vector.tensor_copy(og[:, QG:D], po[:, 1, :])
                nc.sync.dma_start(out[grows, :], og)

            # ---- attention blocks, g-major ----
            def block(g, mc, o_tts):
                cols = slice(g * QG, (g + 1) * QG)
                psY = [
                    ypsum.tile([P, QG], f32, tag="py", name="psY0"),
                    ypsum.tile([P, QG], f32, tag="py", name="psY1"),
                ]
                pts = [None] * NKT

                def s_pair(kc):
                    krows = slice(kc * P, (kc + 1) * P)
                    psS = spsum.tile([P, 2, QG], f32, tag="s")
                    nc.tensor.matmul(
                        psS[:, 0, :],
                        kTz0[:, mc, krows],
                        qT[:, mc, cols],
                        start=True,
                        stop=True,
                    )
                    nc.tensor.matmul(
                        psS[:, 1, :],
                        kTz1[:, mc, krows],
                        qT[:, mc, cols],
                        start=True,
                        stop=True,
                    )
                    pt = ptp.tile([P, 2, QG], bf16, tag="pt")
                    nc.scalar.activation(pt, psS, EXP, scale=0.125)
                    pts[kc] = pt

                def av_pair(kc):
                    for hh in range(2):
                        nc.tensor.matmul(
                            psY[hh][0:65, :],
                            vS[:, kc, 2 * mc + hh, :],
                            pts[kc][:, hh, :],
                            start=(kc == 0),
                            stop=(kc == NKT - 1),
                        )

                # O-projection row-tiles of the previous q-group are slotted
                # into this block's kc loop (their yT is long since final) so
                # they fill tensor slack without starving the ACT pipeline.
                for kc in range(NKT):
                    s_pair(kc)
                    if kc > 0:
                        av_pair(kc - 1)
                    if kc in (5, 11) and o_tts:
                        o_tt(o_tts.pop(0))
                av_pair(NKT - 1)

                # den rows (partition 64) -> f16 -> rank-1 broadcast into
                # partitions 64:128 of the same bank -> reciprocal -> multiply.
                # Casts first so the tensor-queue rank-1s release quickly.
                dsb = (dsb0, dsb1)
                with nc.allow_low_precision(
                    reason="den in fp16 for the rank-1 broadcast; "
                    "den is O(1e2..1e4), fp16 rel err ~5e-4"
                ):
                    nc.vector.tensor_copy(dsb0, psY[0][64:65, :])
                    nc.vector.tensor_copy(dsb1, psY[1][64:65, :])
                for hh in range(2):
                    nc.tensor.matmul(
                        psY[hh][64:128, :],
                        ones1,
                        dsb[hh],
                        start=True,
                        stop=True,
                        skip_group_check=True,
                    )
                for hh in range(2):
                    nc.vector.reciprocal(rbc, psY[hh][64:128, :])
                    nc.vector.tensor_mul(
                        yTg[g][hh * DH : (hh + 1) * DH, mc, :],
                        psY[hh][0:64, :],
                        rbc,
                    )

            for g in range(NQG):
                pend = list(range(4 * (g - 1), 4 * g)) if g > 0 else []
                for mc in range(2):
                    block(g, mc, pend)
            for tt in range(12, 16):
                o_tt(tt)

    _orig_to_json = nc.to_json_bytes
    nc.to_json_bytes = lambda: _split_multiwaits_json(_orig_to_json())
    return nc


def _get_program():
    global _PROGRAM
    if _PROGRAM is None:
        _PROGRAM = _build_program()
    return _PROGRAM


def _in_maps(tgt, memory, W_q, b_q, W_k, b_k, W_v, b_v, W_o):
    bf16 = ml_dtypes.bfloat16
    maps = []
    tT = [np.ascontiguousarray(tgt[b].T).astype(bf16) for b in range(2)]
    mT = [np.ascontiguousarray(memory[b].T).astype(bf16) for b in range(2)]
    for c in range(NCORES):
        b, hg = c // HPC, c % HPC
        sl = slice(hg * 256, (hg + 1) * 256)
        maps.append(
            {
                "tgtT": tT[b],
                "memT": mT[b],
                "wq": np.ascontiguousarray(W_q[:, sl]).astype(bf16),
                "wk": np.ascontiguousarray(W_k[:, sl]).astype(bf16),
                "wv": np.ascontiguousarray(W_v[:, sl]).astype(bf16),
                "wo": np.ascontiguousarray(W_o[sl, :]).astype(bf16),
                "bq": np.ascontiguousarray(b_q[sl]).astype(np.float32),
                "bk": np.ascontiguousarray(b_k[sl]).astype(np.float32),
                "bv": np.ascontiguousarray(b_v[sl]).astype(np.float32),
            }
        )
    return maps


def kernel(tgt, memory, W_q, b_q, W_k, b_k, W_v, b_v, W_o, b_o):
    from concourse.bass_utils import run_bass_kernel_spmd

    tgt = np.asarray(tgt)
    memory = np.asarray(memory)
    nc = _get_program()
    maps = _in_maps(
        np.asarray(tgt, np.float32),
        np.asarray(memory, np.float32),
        np.asarray(W_q, np.float32),
        np.asarray(b_q, np.float32),
        np.asarray(W_k, np.float32),
        np.asarray(b_k, np.float32),
        np.asarray(W_v, np.float32),
        np.asarray(b_v, np.float32),
        np.asarray(W_o, np.float32),
    )
    res = run_bass_kernel_spmd(nc, maps, core_ids=list(range(NCORES)))
    outs = [r["out"] for r in res.results]
    bo = np.asarray(b_o, np.float64)
    full = np.empty((2, T, D), np.float32)
    for b in range(2):
        acc = np.zeros((T, D), np.float64)
        for hg in range(HPC):
            acc += outs[b * HPC + hg].astype(np.float64)
        full[b] = (acc + bo).astype(np.float32)
    return full


# revision 22
# speedup vs baseline: 1.0685x; 1.0650x over previous
"""Cross-attention Trainium2 Bass kernel (v2).

Full inputs in, full output out. Internally: 8-way sharding, data-parallel
over batch (B=2) x tensor-parallel over head groups (16 heads -> 4 groups
of 4). Core c handles batch c//4, head group c%4. Each core computes a
partial output (its 4 heads' contribution through W_o); the host sums the
4 partials per batch and adds b_o.

Device-side structure (v2):
  - All matmuls bf16 (fp32 PSUM accumulate); output DMA'd as fp16 partials.
  - Activations fed pre-transposed (D on partitions).
  - S^T tiles [k x q] via FULL 128-contraction matmuls: the K-stationary
    for each head is zero-padded to 128 rows (kTz0 top-real/bottom-zero,
    kTz1 bottom-real/top-zero) so the moving operand is the shared qT tile.
  - exp batched 2 PSUM banks per ACTIVATE ([128, 2, 512]) to amortize the
    ~352-cycle ACT pipeline overhead.
  - A@V computed as Y^T = V'^T @ P^T: stationary = V' chunk [128k, 65]
    (65th col = ones -> softmax denominators in row 64), moving = exp'd
    score tile [128k, 512q]. Output lands directly in Y^T orientation,
    so no PE transposes. Normalization: DVE reciprocal of the denominator
    row, rank-1 matmul broadcast into partitions 64:128 of the same PSUM
    bank, DVE multiply -> yT (bf16).
  - g-major block loop with just-in-time Q projections; O projection and
    fp16 output DMA interleaved after each q-group.
"""

import numpy as np
import ml_dtypes

T = 2048          # T_dec == T_enc
D = 1024          # d_model
P = 128
HPC = 4           # heads per core
DH = 64           # head dim
KC = D // P       # 8 contraction chunks for projections
NKT = T // P      # 16 key chunks
QG = 512          # q-group width
NQG = T // QG     # 4 q groups
NCORES = 8

_PROGRAM = None


def _split_multiwaits_json(raw: bytes) -> bytes:
    """This walrus build accepts at most ONE sync-wait per instruction.
    Split every multi-wait instruction into single-wait same-engine NoOps
    followed by the instruction (same-engine program order preserves
    semantics exactly)."""
    try:
        import orjson as _json

        loads, dumps = _json.loads, _json.dumps
    except ImportError:
        import json as _json

        loads = _json.loads
        dumps = lambda o: _json.dumps(o).encode()

    j = loads(raw)
    k = 0
    for fn in j["functions"]:
        for bb in fn["blocks"]:
            insts = bb["instructions"]
            out = []
            changed = False
            for inst in insts:
                si = inst.get("sync_info")
                waits = (si.get("on_wait") or []) if si else []
                if len(waits) > 1:
                    for w in waits[:-1]:
                        nop = {
                            "engine": inst["engine"],
                            "ins": [],
                            "outs": [],
                            "name": f"{inst['name']}-sw{k}",
                            "opcode": "NoOp",
                            "sync_info": {"on_update": [], "on_wait": [w]},
                        }
                        if inst.get("debug") is not None:
                            nop["debug"] = inst["debug"]
                        out.append(nop)
                        k += 1
                    si["on_wait"] = [waits[-1]]
                    changed = True
                out.append(inst)
            if changed:
                bb["instructions"] = out
    return dumps(j)


def _build_program():
    import concourse.bass as bass
    import concourse.tile as tile
    import concourse.mybir as mybir
    from concourse.vector_clock import ScopedClock
    from contextlib import ExitStack

    f32 = mybir.dt.float32
    f16 = mybir.dt.float16
    bf16 = mybir.dt.bfloat16
    EXP = mybir.ActivationFunctionType.Exp

    class _TC(tile.TileContext):
        # This walrus build rejects >1 sync waits on the CTRL Drain
        # encoding; split the kernel-tail drain's waits into single-wait
        # SP instructions instead.
        def _drain_and_barrier(self, tick_clock, wait_clock):
            dummy = mybir.InstNoOp(
                name="wait-collector", engine=mybir.EngineType.SP
            )
            wait_clock.add_sem_waits(
                dummy, ScopedClock({None: tick_clock.global_clock})
            )
            si = dummy.sync_info
            waits = list(si.on_wait) if si and si.on_wait else []
            assert self.sems is not None
            by_name = {h.name: h for h in self.sems.allocated().values()}
            for w in waits:
                self.nc.sync.wait_ge(by_name[w.ant_name], w.wait_value)
            self.nc.sync.drain()
            self.nc.all_engine_barrier()
            popped = self.nc._tile_sem_poison_stack.pop()
            assert popped is self._sem_poison
            self.nc.clear_and_free_semaphores(
                list(self.sems.allocated().values())
            )
            self.nc.all_engine_barrier()

    nc = bass.Bass()

    tgtT = nc.dram_tensor("tgtT", [D, T], bf16, kind="ExternalInput")
    memT = nc.dram_tensor("memT", [D, T], bf16, kind="ExternalInput")
    wq = nc.dram_tensor("wq", [D, 256], bf16, kind="ExternalInput")
    wk = nc.dram_tensor("wk", [D, 256], bf16, kind="ExternalInput")
    wv = nc.dram_tensor("wv", [D, 256], bf16, kind="ExternalInput")
    wo = nc.dram_tensor("wo", [256, D], bf16, kind="ExternalInput")
    bq = nc.dram_tensor("bq", [256], f32, kind="ExternalInput")
    bk = nc.dram_tensor("bk", [256], f32, kind="ExternalInput")
    bv = nc.dram_tensor("bv", [256], f32, kind="ExternalInput")
    out = nc.dram_tensor("out", [T, D], f16, kind="ExternalOutput")

    with _TC(nc) as tc, ExitStack() as ctx:
        singles = ctx.enter_context(tc.tile_pool(name="singles", bufs=1))

        # ---- stage A: load weights + transposed activations ----
        # DMA issue order matters for startup: wk+mem first (K/V proj gate
        # the first attention block), tgt + wq later, wo last.
        wk_s = singles.tile([P, KC, 256], bf16, tag="wk")
        # mT/tT as per-q-group quarter tiles so compute deps are quarter-
        # granular (the tile framework tracks writer->reader deps per tile).
        mTq = [
            singles.tile([P, KC, QG], bf16, tag=f"mTq{i}", name=f"mTq{i}")
            for i in range(NQG)
        ]
        wv_s = singles.tile([P, KC, 256], bf16, tag="wv")
        tTq = [
            singles.tile([P, KC, QG], bf16, tag=f"tTq{i}", name=f"tTq{i}")
            for i in range(NQG)
        ]
        wq_s = singles.tile([P, KC, 256], bf16, tag="wq")
        wo_s = singles.tile([P, 2, D], bf16, tag="wo")

        bq_s = singles.tile([P, 2], f32, tag="bq")
        bk_s = singles.tile([P, 2], f32, tag="bk")
        bvb = singles.tile([P, 256], f32, tag="bvb")

        memT_r = memT.rearrange("(c p) t -> p c t", p=P)
        tgtT_r = tgtT.rearrange("(c p) t -> p c t", p=P)

        # Critical loads on the sync-engine HWDGE queue.
        nc.sync.dma_start(wk_s, wk.rearrange("(c p) n -> p c n", p=P))
        nc.sync.dma_start(bk_s, bk.rearrange("(c p) -> p c", p=P))
        for i in range(NQG):
            nc.sync.dma_start(mTq[i], memT_r[:, :, i * QG : (i + 1) * QG])
        nc.sync.dma_start(wv_s, wv.rearrange("(c p) n -> p c n", p=P))
        bv_ap = bass.AP(tensor=bv[:].tensor, offset=0, ap=[[0, P], [1, 256]])
        nc.sync.dma_start(bvb, bv_ap)
        # Deprioritized loads: issued from the scalar engine AFTER a dummy
        # read of mT's last quarter, so they don't steal HBM bandwidth from
        # the loads gating the first matmuls.
        dgate = singles.tile([1, 8], bf16, tag="dgate")
        nc.scalar.copy(dgate, mTq[3][0:1, 0, 0:8])
        for i in range(NQG):
            nc.scalar.dma_start(tTq[i], tgtT_r[:, :, i * QG : (i + 1) * QG])
            if i == 0:
                nc.scalar.dma_start(
                    wq_s, wq.rearrange("(c p) n -> p c n", p=P)
                )
                nc.scalar.dma_start(bq_s, bq.rearrange("(c p) -> p c", p=P))
        nc.scalar.dma_start(wo_s, wo.rearrange("(c p) n -> p c n", p=P))

        # persistent intermediates
        qT = singles.tile([P, 2, T], bf16, tag="qT")    # Q^T (head-pair rows)
        # zero-padded K stationaries: kTz0 rows 0:64 = head0 K^T, rows
        # 64:128 = 0; kTz1 rows 0:64 = 0, rows 64:128 = head1 K^T.
        kTz0 = singles.tile([P, 2, T], bf16, tag="kTz0")
        kTz1 = singles.tile([P, 2, T], bf16, tag="kTz1")
        # V' per key-chunk: [V|1] for every head (k on partitions)
        vS = singles.tile([P, NKT, HPC, DH + 1], bf16, tag="vS")
        # normalized Y^T, split per q-group so O-projection reads of group
        # g-1 don't serialize against normalize writes of group g.
        yTg = [
            singles.tile([P, 2, QG], bf16, tag=f"yTg{i}", name=f"yTg{i}")
            for i in range(NQG)
        ]

        ones1 = singles.tile([1, DH], f16, tag="ones1")  # rank-1 stationary
        dsb0 = singles.tile([1, QG], f16, tag="dsb0")    # den row h0 (f16)
        dsb1 = singles.tile([1, QG], f16, tag="dsb1")    # den row h1 (f16)
        rbc = singles.tile([DH, QG], f32, tag="rbc")     # 1/den broadcast

        nc.vector.memset(vS[:, :, :, DH : DH + 1], 1.0)
        nc.vector.memset(kTz0[64:128, :, :], 0.0)
        nc.vector.memset(kTz1[0:64, :, :], 0.0)
        nc.vector.memset(ones1, 1.0)

        # ---- prologue: all projections (own PSUM pool, closed after) ----
        with tc.tile_pool(name="apsum", bufs=4, space="PSUM") as apsum:

            def k_proj(mc, g):
                cols = slice(g * QG, (g + 1) * QG)
                pk = apsum.tile([P, QG], f32, tag="pa")
                for c in range(KC):
                    nc.tensor.matmul(
                        pk,
                        wk_s[:, c, mc * P : (mc + 1) * P],
                        mTq[g][:, c, :],
                        start=(c == 0),
                        stop=(c == KC - 1),
                    )
                nc.vector.tensor_scalar_add(
                    kTz0[0:64, mc, cols], pk[0:64, :], bk_s[0:64, mc : mc + 1]
                )
                nc.vector.tensor_scalar_add(
                    kTz1[64:128, mc, cols],
                    pk[64:128, :],
                    bk_s[64:128, mc : mc + 1],
                )

            def q_proj(mc, g):
                cols = slice(g * QG, (g + 1) * QG)
                pq = apsum.tile([P, QG], f32, tag="pa")
                for c in range(KC):
                    nc.tensor.matmul(
                        pq,
                        wq_s[:, c, mc * P : (mc + 1) * P],
                        tTq[g][:, c, :],
                        start=(c == 0),
                        stop=(c == KC - 1),
                    )
                nc.vector.tensor_scalar_add(
                    qT[:, mc, cols], pq, bq_s[:, mc : mc + 1]
                )

            def v_proj(tt):
                pv = apsum.tile([P, QG], f32, tag="pa")
                for c in range(KC):
                    nc.tensor.matmul(
                        pv[:, 0:256],
                        mTq[tt // 4][:, c, (tt % 4) * P : (tt % 4 + 1) * P],
                        wv_s[:, c, :],
                        start=(c == 0),
                        stop=(c == KC - 1),
                    )
                pv4 = pv[:, 0:256].rearrange("p (h d) -> p h d", h=HPC)
                bvb4 = bvb.rearrange("p (h d) -> p h d", h=HPC)
                nc.vector.tensor_add(vS[:, tt, :, 0:DH], pv4, bvb4)

            # K/V interleaved with mT quarter arrival.
            k_proj(0, 0)
            k_proj(0, 1)
            for tt in range(8):
                v_proj(tt)
            k_proj(0, 2)
            k_proj(0, 3)
            for tt in range(8, NKT):
                v_proj(tt)
            for g in range(NQG):
                k_proj(1, g)
            for g in range(NQG):
                for mc in range(2):
                    q_proj(mc, g)

        # ---- attention (prologue PSUM banks freed: 4+4 of 8 used) ----
        with (
            tc.tile_pool(name="spsum", bufs=2, space="PSUM") as spsum,
            tc.tile_pool(name="ypsum", bufs=4, space="PSUM") as ypsum,
            tc.tile_pool(name="ostg", bufs=3) as ostg,
            tc.tile_pool(name="ptp", bufs=6) as ptp,
        ):

            def o_tt(tt):
                qrows = slice((tt % 4) * P, (tt % 4 + 1) * P)
                grows = slice(tt * P, (tt + 1) * P)
                po = spsum.tile([P, 2, QG], f32, tag="s")
                for ng in range(2):
                    ncols = slice(ng * QG, (ng + 1) * QG)
                    for mc in range(2):
                        nc.tensor.matmul(
                            po[:, ng, :],
                            yTg[tt // 4][:, mc, qrows],
                            wo_s[:, mc, ncols],
                            start=(mc == 0),
                            stop=(mc == 1),
                        )
                og = ostg.tile([P, D], f16, tag="og")
                nc.vector.tensor_copy(og[:, 0:QG], po[:, 0, :])
                nc.vector.tensor_copy(og[:, QG:D], po[:, 1, :])
                nc.sync.dma_start(out[grows, :], og)

            # ---- attention blocks, g-major ----
            def block(g, mc, o_tts):
                cols = slice(g * QG, (g + 1) * QG)
                psY = [
                    ypsum.tile([P, QG], f32, tag="py", name="psY0"),
                    ypsum.tile([P, QG], f32, tag="py", name="psY1"),
                ]
                pts = [None] * NKT

                def s_pair(kc):
                    krows = slice(kc * P, (kc + 1) * P)
                    psS = spsum.tile([P, 2, QG], f32, tag="s")
                    nc.tensor.matmul(
                        psS[:, 0, :],
                        kTz0[:, mc, krows],
                        qT[:, mc, cols],
                        start=True,
                        stop=True,
                    )
                    nc.tensor.matmul(
                        psS[:, 1, :],
                        kTz1[:, mc, krows],
                        qT[:, mc, cols],
                        start=True,
                        stop=True,
                    )
                    pt = ptp.tile([P, 2, QG], bf16, tag="pt")
                    nc.scalar.activation(pt, psS, EXP, scale=0.125)
                    pts[kc] = pt

                def av_pair(kc):
                    for hh in range(2):
                        nc.tensor.matmul(
                            psY[hh][0:65, :],
                            vS[:, kc, 2 * mc + hh, :],
                            pts[kc][:, hh, :],
                            start=(kc == 0),
                            stop=(kc == NKT - 1),
                        )

                # O-projection row-tiles of the previous q-group are slotted
                # into this block's kc loop (their yT is long since final) so
                # they fill tensor slack without starving the ACT pipeline.
                for kc in range(NKT):
                    s_pair(kc)
                    if kc > 0:
                        av_pair(kc - 1)
                    if kc in (9, 13) and o_tts:
                        o_tt(o_tts.pop(0))
                av_pair(NKT - 1)

                # den rows (partition 64) -> f16 -> rank-1 broadcast into
                # partitions 64:128 of the same bank -> reciprocal -> multiply.
                # Casts first so the tensor-queue rank-1s release quickly.
                dsb = (dsb0, dsb1)
                with nc.allow_low_precision(
                    reason="den in fp16 for the rank-1 broadcast; "
                    "den is O(1e2..1e4), fp16 rel err ~5e-4"
                ):
                    nc.vector.tensor_copy(dsb0, psY[0][64:65, :])
                    nc.vector.tensor_copy(dsb1, psY[1][64:65, :])
                for hh in range(2):
                    nc.tensor.matmul(
                        psY[hh][64:128, :],
                        ones1,
                        dsb[hh],
                        start=True,
                        stop=True,
                        skip_group_check=True,
                    )
                for hh in range(2):
                    nc.vector.reciprocal(rbc, psY[hh][64:128, :])
                    nc.vector.tensor_mul(
                        yTg[g][hh * DH : (hh + 1) * DH, mc, :],
                        psY[hh][0:64, :],
                        rbc,
                    )

            for g in range(NQG):
                pend = list(range(4 * (g - 1), 4 * g)) if g > 0 else []
                for mc in range(2):
                    block(g, mc, pend)
            for tt in range(12, 16):
                o_tt(tt)

    _orig_to_json = nc.to_json_bytes
    nc.to_json_bytes = lambda: _split_multiwaits_json(_orig_to_json())
    return nc


def _get_program():
    global _PROGRAM
    if _PROGRAM is None:
        _PROGRAM = _build_program()
    return _PROGRAM


def _in_maps(tgt, memory, W_q, b_q, W_k, b_k, W_v, b_v, W_o):
    bf16 = ml_dtypes.bfloat16
    maps = []
    tT = [np.ascontiguousarray(tgt[b].T).astype(bf16) for b in range(2)]
    mT = [np.ascontiguousarray(memory[b].T).astype(bf16) for b in range(2)]
    for c in range(NCORES):
        b, hg = c // HPC, c % HPC
        sl = slice(hg * 256, (hg + 1) * 256)
        maps.append(
            {
                "tgtT": tT[b],
                "memT": mT[b],
                "wq": np.ascontiguousarray(W_q[:, sl]).astype(bf16),
                "wk": np.ascontiguousarray(W_k[:, sl]).astype(bf16),
                "wv": np.ascontiguousarray(W_v[:, sl]).astype(bf16),
                "wo": np.ascontiguousarray(W_o[sl, :]).astype(bf16),
                "bq": np.ascontiguousarray(b_q[sl]).astype(np.float32),
                "bk": np.ascontiguousarray(b_k[sl]).astype(np.float32),
                "bv": np.ascontiguousarray(b_v[sl]).astype(np.float32),
            }
        )
    return maps


def kernel(tgt, memory, W_q, b_q, W_k, b_k, W_v, b_v, W_o, b_o):
    from concourse.bass_utils import run_bass_kernel_spmd

    tgt = np.asarray(tgt)
    memory = np.asarray(memory)
    nc = _get_program()
    maps = _in_maps(
        np.asarray(tgt, np.float32),
        np.asarray(memory, np.float32),
        np.asarray(W_q, np.float32),
        np.asarray(b_q, np.float32),
        np.asarray(W_k, np.float32),
        np.asarray(b_k, np.float32),
        np.asarray(W_v, np.float32),
        np.asarray(b_v, np.float32),
        np.asarray(W_o, np.float32),
    )
    res = run_bass_kernel_spmd(nc, maps, core_ids=list(range(NCORES)))
    outs = [r["out"] for r in res.results]
    bo = np.asarray(b_o, np.float64)
    full = np.empty((2, T, D), np.float32)
    for b in range(2):
        acc = np.zeros((T, D), np.float64)
        for hg in range(HPC):
            acc += outs[b * HPC + hg].astype(np.float64)
        full[b] = (acc + bo).astype(np.float32)
    return full


# revision 23
# speedup vs baseline: 1.1400x; 1.0669x over previous
"""Cross-attention Trainium2 Bass kernel (v2).

Full inputs in, full output out. Internally: 8-way sharding, data-parallel
over batch (B=2) x tensor-parallel over head groups (16 heads -> 4 groups
of 4). Core c handles batch c//4, head group c%4. Each core computes a
partial output (its 4 heads' contribution through W_o); the host sums the
4 partials per batch and adds b_o.

Device-side structure (v2):
  - All matmuls bf16 (fp32 PSUM accumulate); output DMA'd as fp16 partials.
  - Activations fed pre-transposed (D on partitions).
  - S^T tiles [k x q] via FULL 128-contraction matmuls: the K-stationary
    for each head is zero-padded to 128 rows (kTz0 top-real/bottom-zero,
    kTz1 bottom-real/top-zero) so the moving operand is the shared qT tile.
  - exp batched 2 PSUM banks per ACTIVATE ([128, 2, 512]) to amortize the
    ~352-cycle ACT pipeline overhead.
  - A@V computed as Y^T = V'^T @ P^T: stationary = V' chunk [128k, 65]
    (65th col = ones -> softmax denominators in row 64), moving = exp'd
    score tile [128k, 512q]. Output lands directly in Y^T orientation,
    so no PE transposes. Normalization: DVE reciprocal of the denominator
    row, rank-1 matmul broadcast into partitions 64:128 of the same PSUM
    bank, DVE multiply -> yT (bf16).
  - g-major block loop with just-in-time Q projections; O projection and
    fp16 output DMA interleaved after each q-group.
"""

import numpy as np
import ml_dtypes

T = 2048          # T_dec == T_enc
D = 1024          # d_model
P = 128
HPC = 4           # heads per core
DH = 64           # head dim
KC = D // P       # 8 contraction chunks for projections
NKT = T // P      # 16 key chunks
QG = 512          # q-group width
NQG = T // QG     # 4 q groups
NCORES = 8

_PROGRAM = None


def _split_multiwaits_json(raw: bytes) -> bytes:
    """This walrus build accepts at most ONE sync-wait per instruction.
    Split every multi-wait instruction into single-wait same-engine NoOps
    followed by the instruction (same-engine program order preserves
    semantics exactly)."""
    try:
        import orjson as _json

        loads, dumps = _json.loads, _json.dumps
    except ImportError:
        import json as _json

        loads = _json.loads
        dumps = lambda o: _json.dumps(o).encode()

    j = loads(raw)
    k = 0
    for fn in j["functions"]:
        for bb in fn["blocks"]:
            insts = bb["instructions"]
            out = []
            changed = False
            for inst in insts:
                si = inst.get("sync_info")
                waits = (si.get("on_wait") or []) if si else []
                if len(waits) > 1:
                    for w in waits[:-1]:
                        nop = {
                            "engine": inst["engine"],
                            "ins": [],
                            "outs": [],
                            "name": f"{inst['name']}-sw{k}",
                            "opcode": "NoOp",
                            "sync_info": {"on_update": [], "on_wait": [w]},
                        }
                        if inst.get("debug") is not None:
                            nop["debug"] = inst["debug"]
                        out.append(nop)
                        k += 1
                    si["on_wait"] = [waits[-1]]
                    changed = True
                out.append(inst)
            if changed:
                bb["instructions"] = out
    return dumps(j)


def _build_program():
    import concourse.bass as bass
    import concourse.tile as tile
    import concourse.mybir as mybir
    from concourse.vector_clock import ScopedClock
    from contextlib import ExitStack

    f32 = mybir.dt.float32
    f16 = mybir.dt.float16
    bf16 = mybir.dt.bfloat16
    EXP = mybir.ActivationFunctionType.Exp

    class _TC(tile.TileContext):
        # This walrus build rejects >1 sync waits on the CTRL Drain
        # encoding; split the kernel-tail drain's waits into single-wait
        # SP instructions instead.
        def _drain_and_barrier(self, tick_clock, wait_clock):
            dummy = mybir.InstNoOp(
                name="wait-collector", engine=mybir.EngineType.SP
            )
            wait_clock.add_sem_waits(
                dummy, ScopedClock({None: tick_clock.global_clock})
            )
            si = dummy.sync_info
            waits = list(si.on_wait) if si and si.on_wait else []
            assert self.sems is not None
            by_name = {h.name: h for h in self.sems.allocated().values()}
            for w in waits:
                self.nc.sync.wait_ge(by_name[w.ant_name], w.wait_value)
            self.nc.sync.drain()
            self.nc.all_engine_barrier()
            popped = self.nc._tile_sem_poison_stack.pop()
            assert popped is self._sem_poison
            self.nc.clear_and_free_semaphores(
                list(self.sems.allocated().values())
            )
            self.nc.all_engine_barrier()

    nc = bass.Bass()

    tgtT = nc.dram_tensor("tgtT", [D, T], bf16, kind="ExternalInput")
    memT = nc.dram_tensor("memT", [D, T], bf16, kind="ExternalInput")
    wq = nc.dram_tensor("wq", [D, 256], bf16, kind="ExternalInput")
    wk = nc.dram_tensor("wk", [D, 256], bf16, kind="ExternalInput")
    wv = nc.dram_tensor("wv", [D, 256], bf16, kind="ExternalInput")
    wo = nc.dram_tensor("wo", [256, D], bf16, kind="ExternalInput")
    bq = nc.dram_tensor("bq", [256], f32, kind="ExternalInput")
    bk = nc.dram_tensor("bk", [256], f32, kind="ExternalInput")
    bv = nc.dram_tensor("bv", [256], f32, kind="ExternalInput")
    out = nc.dram_tensor("out", [T, D], f16, kind="ExternalOutput")

    with _TC(nc) as tc, ExitStack() as ctx:
        singles = ctx.enter_context(tc.tile_pool(name="singles", bufs=1))

        # ---- stage A: load weights + transposed activations ----
        # DMA issue order matters for startup: wk+mem first (K/V proj gate
        # the first attention block), tgt + wq later, wo last.
        wk_s = singles.tile([P, KC, 256], bf16, tag="wk")
        # mT/tT as per-q-group quarter tiles so compute deps are quarter-
        # granular (the tile framework tracks writer->reader deps per tile).
        mTq = [
            singles.tile([P, KC, QG], bf16, tag=f"mTq{i}", name=f"mTq{i}")
            for i in range(NQG)
        ]
        wv_s = singles.tile([P, KC, 256], bf16, tag="wv")
        tTq = [
            singles.tile([P, KC, QG], bf16, tag=f"tTq{i}", name=f"tTq{i}")
            for i in range(NQG)
        ]
        wq_s = singles.tile([P, KC, 256], bf16, tag="wq")
        wo_s = singles.tile([P, 2, D], bf16, tag="wo")

        bq_s = singles.tile([P, 2], f32, tag="bq")
        bk_s = singles.tile([P, 2], f32, tag="bk")
        bvb = singles.tile([P, 256], f32, tag="bvb")

        memT_r = memT.rearrange("(c p) t -> p c t", p=P)
        tgtT_r = tgtT.rearrange("(c p) t -> p c t", p=P)

        # All input loads on the sync-engine HWDGE queue: within one queue
        # transfers are FIFO, so issue order IS priority order (observed to
        # sustain full HBM bandwidth on one queue).
        bv_ap = bass.AP(tensor=bv[:].tensor, offset=0, ap=[[0, P], [1, 256]])
        nc.sync.dma_start(wk_s, wk.rearrange("(c p) n -> p c n", p=P))
        nc.sync.dma_start(bk_s, bk.rearrange("(c p) -> p c", p=P))
        nc.sync.dma_start(mTq[0], memT_r[:, :, 0:QG])
        nc.sync.dma_start(wv_s, wv.rearrange("(c p) n -> p c n", p=P))
        nc.sync.dma_start(bvb, bv_ap)
        for i in range(1, NQG):
            nc.sync.dma_start(mTq[i], memT_r[:, :, i * QG : (i + 1) * QG])
        nc.sync.dma_start(tTq[0], tgtT_r[:, :, 0:QG])
        nc.sync.dma_start(wq_s, wq.rearrange("(c p) n -> p c n", p=P))
        nc.sync.dma_start(bq_s, bq.rearrange("(c p) -> p c", p=P))
        for i in range(1, NQG):
            nc.sync.dma_start(tTq[i], tgtT_r[:, :, i * QG : (i + 1) * QG])
        nc.sync.dma_start(wo_s, wo.rearrange("(c p) n -> p c n", p=P))

        # persistent intermediates
        qT = singles.tile([P, 2, T], bf16, tag="qT")    # Q^T (head-pair rows)
        # zero-padded K stationaries: kTz0 rows 0:64 = head0 K^T, rows
        # 64:128 = 0; kTz1 rows 0:64 = 0, rows 64:128 = head1 K^T.
        kTz0 = singles.tile([P, 2, T], bf16, tag="kTz0")
        kTz1 = singles.tile([P, 2, T], bf16, tag="kTz1")
        # V' per key-chunk: [V|1] for every head (k on partitions)
        vS = singles.tile([P, NKT, HPC, DH + 1], bf16, tag="vS")
        # normalized Y^T, split per q-group so O-projection reads of group
        # g-1 don't serialize against normalize writes of group g.
        yTg = [
            singles.tile([P, 2, QG], bf16, tag=f"yTg{i}", name=f"yTg{i}")
            for i in range(NQG)
        ]

        ones1 = singles.tile([1, DH], f16, tag="ones1")  # rank-1 stationary
        dsb0 = singles.tile([1, QG], f16, tag="dsb0")    # den row h0 (f16)
        dsb1 = singles.tile([1, QG], f16, tag="dsb1")    # den row h1 (f16)
        rbc = singles.tile([DH, QG], f32, tag="rbc")     # 1/den broadcast

        nc.vector.memset(vS[:, :, :, DH : DH + 1], 1.0)
        nc.vector.memset(kTz0[64:128, :, :], 0.0)
        nc.vector.memset(kTz1[0:64, :, :], 0.0)
        nc.vector.memset(ones1, 1.0)

        # ---- prologue: all projections (own PSUM pool, closed after) ----
        with tc.tile_pool(name="apsum", bufs=4, space="PSUM") as apsum:

            def k_proj(mc, g):
                cols = slice(g * QG, (g + 1) * QG)
                pk = apsum.tile([P, QG], f32, tag="pa")
                for c in range(KC):
                    nc.tensor.matmul(
                        pk,
                        wk_s[:, c, mc * P : (mc + 1) * P],
                        mTq[g][:, c, :],
                        start=(c == 0),
                        stop=(c == KC - 1),
                    )
                nc.vector.tensor_scalar_add(
                    kTz0[0:64, mc, cols], pk[0:64, :], bk_s[0:64, mc : mc + 1]
                )
                nc.vector.tensor_scalar_add(
                    kTz1[64:128, mc, cols],
                    pk[64:128, :],
                    bk_s[64:128, mc : mc + 1],
                )

            def q_proj(mc, g):
                cols = slice(g * QG, (g + 1) * QG)
                pq = apsum.tile([P, QG], f32, tag="pa")
                for c in range(KC):
                    nc.tensor.matmul(
                        pq,
                        wq_s[:, c, mc * P : (mc + 1) * P],
                        tTq[g][:, c, :],
                        start=(c == 0),
                        stop=(c == KC - 1),
                    )
                nc.vector.tensor_scalar_add(
                    qT[:, mc, cols], pq, bq_s[:, mc : mc + 1]
                )

            def v_proj(tt):
                pv = apsum.tile([P, QG], f32, tag="pa")
                for c in range(KC):
                    nc.tensor.matmul(
                        pv[:, 0:256],
                        mTq[tt // 4][:, c, (tt % 4) * P : (tt % 4 + 1) * P],
                        wv_s[:, c, :],
                        start=(c == 0),
                        stop=(c == KC - 1),
                    )
                pv4 = pv[:, 0:256].rearrange("p (h d) -> p h d", h=HPC)
                bvb4 = bvb.rearrange("p (h d) -> p h d", h=HPC)
                nc.vector.tensor_add(vS[:, tt, :, 0:DH], pv4, bvb4)

            # K/V interleaved with mT quarter arrival.
            k_proj(0, 0)
            k_proj(0, 1)
            for tt in range(8):
                v_proj(tt)
            k_proj(0, 2)
            k_proj(0, 3)
            for tt in range(8, NKT):
                v_proj(tt)
            for g in range(NQG):
                k_proj(1, g)
            for g in range(NQG):
                for mc in range(2):
                    q_proj(mc, g)

        # ---- attention (prologue PSUM banks freed: 4+4 of 8 used) ----
        with (
            tc.tile_pool(name="spsum", bufs=2, space="PSUM") as spsum,
            tc.tile_pool(name="ypsum", bufs=4, space="PSUM") as ypsum,
            tc.tile_pool(name="ostg", bufs=3) as ostg,
            tc.tile_pool(name="ptp", bufs=6) as ptp,
        ):

            def o_tt(tt):
                qrows = slice((tt % 4) * P, (tt % 4 + 1) * P)
                grows = slice(tt * P, (tt + 1) * P)
                po = spsum.tile([P, 2, QG], f32, tag="s")
                for ng in range(2):
                    ncols = slice(ng * QG, (ng + 1) * QG)
                    for mc in range(2):
                        nc.tensor.matmul(
                            po[:, ng, :],
                            yTg[tt // 4][:, mc, qrows],
                            wo_s[:, mc, ncols],
                            start=(mc == 0),
                            stop=(mc == 1),
                        )
                og = ostg.tile([P, D], f16, tag="og")
                nc.vector.tensor_copy(og[:, 0:QG], po[:, 0, :])
                nc.vector.tensor_copy(og[:, QG:D], po[:, 1, :])
                nc.sync.dma_start(out[grows, :], og)

            # ---- attention blocks, g-major ----
            def block(g, mc, o_tts):
                cols = slice(g * QG, (g + 1) * QG)
                psY = [
                    ypsum.tile([P, QG], f32, tag="py", name="psY0"),
                    ypsum.tile([P, QG], f32, tag="py", name="psY1"),
                ]
                pts = [None] * NKT

                def s_pair(kc):
                    krows = slice(kc * P, (kc + 1) * P)
                    psS = spsum.tile([P, 2, QG], f32, tag="s")
                    nc.tensor.matmul(
                        psS[:, 0, :],
                        kTz0[:, mc, krows],
                        qT[:, mc, cols],
                        start=True,
                        stop=True,
                    )
                    nc.tensor.matmul(
                        psS[:, 1, :],
                        kTz1[:, mc, krows],
                        qT[:, mc, cols],
                        start=True,
                        stop=True,
                    )
                    pt = ptp.tile([P, 2, QG], bf16, tag="pt")
                    nc.scalar.activation(pt, psS, EXP, scale=0.125)
                    pts[kc] = pt

                def av_pair(kc):
                    for hh in range(2):
                        nc.tensor.matmul(
                            psY[hh][0:65, :],
                            vS[:, kc, 2 * mc + hh, :],
                            pts[kc][:, hh, :],
                            start=(kc == 0),
                            stop=(kc == NKT - 1),
                        )

                # O-projection row-tiles of the previous q-group are slotted
                # into this block's kc loop (their yT is long since final) so
                # they fill tensor slack without starving the ACT pipeline.
                for kc in range(NKT):
                    s_pair(kc)
                    if kc > 0:
                        av_pair(kc - 1)
                    if kc in (9, 13) and o_tts:
                        o_tt(o_tts.pop(0))
                av_pair(NKT - 1)

                # den rows (partition 64) -> f16 -> rank-1 broadcast into
                # partitions 64:128 of the same bank -> reciprocal -> multiply.
                # Casts first so the tensor-queue rank-1s release quickly.
                dsb = (dsb0, dsb1)
                with nc.allow_low_precision(
                    reason="den in fp16 for the rank-1 broadcast; "
                    "den is O(1e2..1e4), fp16 rel err ~5e-4"
                ):
                    nc.vector.tensor_copy(dsb0, psY[0][64:65, :])
                    nc.vector.tensor_copy(dsb1, psY[1][64:65, :])
                for hh in range(2):
                    nc.tensor.matmul(
                        psY[hh][64:128, :],
                        ones1,
                        dsb[hh],
                        start=True,
                        stop=True,
                        skip_group_check=True,
                    )
                for hh in range(2):
                    nc.vector.reciprocal(rbc, psY[hh][64:128, :])
                    nc.vector.tensor_mul(
                        yTg[g][hh * DH : (hh + 1) * DH, mc, :],
                        psY[hh][0:64, :],
                        rbc,
                    )

            for g in range(NQG):
                pend = list(range(4 * (g - 1), 4 * g)) if g > 0 else []
                for mc in range(2):
                    block(g, mc, pend)
            for tt in range(12, 16):
                o_tt(tt)

    _orig_to_json = nc.to_json_bytes
    nc.to_json_bytes = lambda: _split_multiwaits_json(_orig_to_json())
    return nc


def _get_program():
    global _PROGRAM
    if _PROGRAM is None:
        _PROGRAM = _build_program()
    return _PROGRAM


def _in_maps(tgt, memory, W_q, b_q, W_k, b_k, W_v, b_v, W_o):
    bf16 = ml_dtypes.bfloat16
    maps = []
    tT = [np.ascontiguousarray(tgt[b].T).astype(bf16) for b in range(2)]
    mT = [np.ascontiguousarray(memory[b].T).astype(bf16) for b in range(2)]
    for c in range(NCORES):
        b, hg = c // HPC, c % HPC
        sl = slice(hg * 256, (hg + 1) * 256)
        maps.append(
            {
                "tgtT": tT[b],
                "memT": mT[b],
                "wq": np.ascontiguousarray(W_q[:, sl]).astype(bf16),
                "wk": np.ascontiguousarray(W_k[:, sl]).astype(bf16),
                "wv": np.ascontiguousarray(W_v[:, sl]).astype(bf16),
                "wo": np.ascontiguousarray(W_o[sl, :]).astype(bf16),
                "bq": np.ascontiguousarray(b_q[sl]).astype(np.float32),
                "bk": np.ascontiguousarray(b_k[sl]).astype(np.float32),
                "bv": np.ascontiguousarray(b_v[sl]).astype(np.float32),
            }
        )
    return maps


def kernel(tgt, memory, W_q, b_q, W_k, b_k, W_v, b_v, W_o, b_o):
    from concourse.bass_utils import run_bass_kernel_spmd

    tgt = np.asarray(tgt)
    memory = np.asarray(memory)
    nc = _get_program()
    maps = _in_maps(
        np.asarray(tgt, np.float32),
        np.asarray(memory, np.float32),
        np.asarray(W_q, np.float32),
        np.asarray(b_q, np.float32),
        np.asarray(W_k, np.float32),
        np.asarray(b_k, np.float32),
        np.asarray(W_v, np.float32),
        np.asarray(b_v, np.float32),
        np.asarray(W_o, np.float32),
    )
    res = run_bass_kernel_spmd(nc, maps, core_ids=list(range(NCORES)))
    outs = [r["out"] for r in res.results]
    bo = np.asarray(b_o, np.float64)
    full = np.empty((2, T, D), np.float32)
    for b in range(2):
        acc = np.zeros((T, D), np.float64)
        for hg in range(HPC):
            acc += outs[b * HPC + hg].astype(np.float64)
        full[b] = (acc + bo).astype(np.float32)
    return full
